# revision 1
# baseline (speedup 1.0000x reference)
"""Trainium2 Bass kernel v2: ragged phonology-embedding mean + position add.

Reference semantics (per (b, s)):
    out[b, s, :] = mean_{g < len[b,s]} table[tok[b,s,g], :] + pos[s, :]

Data-parallel over B across 8 cores; per core 16 output tiles of 128 rows.
Three vocab regions, chosen to balance PE (matmul) vs Q7 (gather desc-gen):

  - FIAT  (f chunks): vocab rows [0, 128f) loaded contiguously (no gather);
    every tile matmuls them one-hot style. Pure PE work available from ~6us,
    covering the fixed ~16us Q7 ucode-library load.
  - HOT   (h chunks): the 128h most popular remaining rows (by number of
    tiles referencing them), gathered ONCE per core into SBUF; every tile
    matmuls all h chunks. Absorbs ~6-9 uses per gathered index vs ~1.2 for
    cold rows, cutting Q7 descriptor-generation time.
  - COLD: per-tile leftovers, deduped and gathered per tile-GROUP (2 or 4
    tiles) with membership-ordered unions so most chunks matmul into only
    1-2 PSUM tiles.

Per tile: PSUM group 1 = fiat chunks, evicted by VectorE (+pos -> bf16) to
an SBUF accumulator so PSUM never blocks on gather arrival; PSUM group 2 =
hot + cold chunks, finished by a second VectorE add (psum + accumulator)
-> bf16 -> HBM. Weights W[p, m] = count/len make the ragged mean a matmul.
A host-side makespan simulator grid-searches (f, h, group size).
"""

import os
import numpy as np
import ml_dtypes

import concourse.bass as bass
import concourse.bacc as bacc
import concourse.mybir as mybir
import concourse.tile as tile
from concourse.bass_utils import run_bass_kernel_spmd

B, S, G = 128, 128, 8
VOCAB, D = 2048, 1024
NCORES = 8
BPC = B // NCORES
R = BPC * S
P = 128
NT = R // P                 # 16 output tiles per core
KT = VOCAB // P
MAXCH = 8                   # dma_gather HW cap: 1024 indices per call

# calibrated engine costs (ns), from HW traces
Q7_LIB_END_NS = 21_800.0    # preamble + ucode library load
Q7_NS_PER_IDX = 8.75
Q7_CALL_FIXED_NS = 700.0
PE_START_NS = 10_000.0
PE_NS_PER_CHUNK = 460.0     # [128x128] @ [128x1024] as 2 N=512 matmuls
DVE_PASS_NS = 1_220.0
DRAIN_NS = 1_500.0          # gather-call SDMA drain before matmul can start
TEARDOWN_NS = 9_500.0
GT_BUFS = 5


def _cdiv(a, b):
    return -(-a // b)


def _base_stats(phon_tokens, group_len_raw):
    toks = np.asarray(phon_tokens).astype(np.int64).reshape(B, S, G)
    lens = (np.asarray(group_len_raw).astype(np.int64) + 1).reshape(B, S)
    toks_c = toks.reshape(NCORES, R, G)
    lens_c = lens.reshape(NCORES, R)
    uniqs = {}
    wmats = {}
    kvs = np.zeros((NCORES, VOCAB), np.int64)
    for c in range(NCORES):
        for t in range(NT):
            tl = toks_c[c, t * P:(t + 1) * P]
            ll = lens_c[c, t * P:(t + 1) * P]
            valid = np.arange(G)[None, :] < ll[:, None]
            flat = tl[valid]
            pair = np.repeat(np.arange(P), ll)
            uniq, inv = np.unique(flat, return_inverse=True)
            wm = np.zeros((uniq.size, P), np.float32)
            np.add.at(wm, (inv, pair), 1.0 / ll[pair])
            uniqs[c, t] = uniq
            wmats[c, t] = wm
            kvs[c, uniq] += 1
    return uniqs, wmats, kvs


def _plan(uniqs, kvs, f, h, gsz):
    """Build regions/groups/calls; return plan dict + per-core idx lists."""
    nfiat = 128 * f
    hot_rows = {}
    for c in range(NCORES):
        kv = kvs[c].copy()
        kv[:nfiat] = -1
        if h > 0:
            order = np.lexsort((np.arange(VOCAB), -kv))
            hot_rows[c] = np.sort(order[:128 * h])
        else:
            hot_rows[c] = np.zeros(0, np.int64)

    colds = {}
    for c in range(NCORES):
        hotset = hot_rows[c]
        for t in range(NT):
            u = uniqs[c, t]
            mask = u >= nfiat
            if h > 0:
                mask &= ~np.isin(u, hotset, assume_unique=True)
            colds[c, t] = u[mask]

    # build one segment per tile-group, then pack segments into calls
    ngroups = NT // gsz
    segments = []
    for gi in range(ngroups):
        tiles = list(range(gi * gsz, (gi + 1) * gsz))
        nU = np.zeros(NCORES, int)
        tok = {}
        msk = {}
        for c in range(NCORES):
            sets = [colds[c, t] for t in tiles]
            union = np.unique(np.concatenate(sets))
            m = np.zeros(union.size, np.int64)
            for bi, s in enumerate(sets):
                m[np.isin(union, s, assume_unique=True)] |= 1 << bi
            # ordering key: cluster each tile's tokens into few runs
            if gsz == 2:
                key = np.select([m == 1, m == 3, m == 2], [0, 1, 2])
            else:
                lowm = m & 3
                highm = (m >> 2) & 3
                blk = np.where(highm == 0, 0, np.where(lowm == 0, 2, 1))
                sub = np.select([m == 1, m == 3, m == 2], [0, 1, 2], 0)
                sub2 = np.select([m == 4, m == 12, m == 8], [0, 1, 2], 0)
                key = blk * 16 + np.where(blk == 0, sub, np.where(
                    blk == 2, sub2, m))
            order = np.lexsort((union, key))
            tok[c] = union[order]
            msk[c] = m[order]
            nU[c] = union.size
        nch = max(int(_cdiv(int(nU.max()), P)), 1)
        assert nch <= MAXCH, (nch, gi)
        segments.append(dict(tiles=tiles, tok=tok, msk=msk, nch=nch))

    # pack: first two calls fine-grained (1 segment) for early PE data,
    # then as many segments as fit in MAXCH chunks per call.
    import os as _os
    merge = _os.environ.get("PACK", "2") != "1"
    packs = []
    i = 0
    while i < len(segments):
        cur = [segments[i]]
        nch = segments[i]["nch"]
        i += 1
        # merge middle calls; keep the first and last calls single-segment
        # so PE gets data early and the tail stays fine-grained
        if merge and len(packs) >= 1 and i < len(segments) - 1:
            while i < len(segments) - 1 and nch + segments[i]["nch"] <= MAXCH:
                nch += segments[i]["nch"]
                cur.append(segments[i])
                i += 1
        packs.append(cur)

    calls = []
    tokpad = {}   # (core, call_idx) -> padded int64 [nch*P]
    tokval = {}   # (core, call_idx) -> bool valid mask [nch*P]
    chunk_off = h
    entry_off = 0
    for pack in packs:
        gidx = len(calls)
        nch = sum(s["nch"] for s in pack)
        tiles = [t for s in pack for t in s["tiles"]]
        ent = set()
        for c in range(NCORES):
            tp = np.zeros(nch * P, np.int64)
            tv = np.zeros(nch * P, bool)
            off = 0
            for s in pack:
                tl = s["tok"][c]
                tp[off:off + tl.size] = tl
                tv[off:off + tl.size] = True
                m = s["msk"][c]
                for j in range(_cdiv(m.size, P)):
                    seg = m[j * P:(j + 1) * P]
                    gj = off // P + j
                    for bi, t in enumerate(s["tiles"]):
                        if np.any(seg & (1 << bi)):
                            ent.add((gj, t))
                off += s["nch"] * P
            tokpad[c, gidx] = tp
            tokval[c, gidx] = tv
        entries = sorted(ent)
        calls.append(dict(
            nch=nch, idx_base=chunk_off, entry_base=entry_off,
            entries=entries, grp=tuple(tiles),
        ))
        chunk_off += nch
        entry_off += len(entries)

    # split the final call at a chunk boundary so the post-gather tail is
    # short (padded representation splits cleanly)
    last = calls[-1]
    if last["nch"] >= 4:
        calls.pop()
        nch = last["nch"]
        k = nch - 2
        gidx0 = len(calls)
        ents1 = [(j, t) for (j, t) in last["entries"] if j < k]
        ents2 = [(j - k, t) for (j, t) in last["entries"] if j >= k]
        calls.append(dict(
            nch=k, idx_base=last["idx_base"], entry_base=last["entry_base"],
            entries=ents1, grp=last["grp"]))
        calls.append(dict(
            nch=nch - k, idx_base=last["idx_base"] + k,
            entry_base=last["entry_base"] + len(ents1),
            entries=ents2, grp=last["grp"]))
        for c in range(NCORES):
            tp = tokpad.pop((c, gidx0))
            tv = tokval.pop((c, gidx0))
            tokpad[c, gidx0] = tp[:k * P]
            tokval[c, gidx0] = tv[:k * P]
            tokpad[c, gidx0 + 1] = tp[k * P:]
            tokval[c, gidx0 + 1] = tv[k * P:]

    gfirst, glast = {}, {}
    for ci, call in enumerate(calls):
        for e, (j, t) in enumerate(call["entries"]):
            gfirst.setdefault(t, (ci, e))
            glast[t] = (ci, e)
    assert len(gfirst) == NT
    for ci, call in enumerate(calls):
        call["first"] = {t: e for t, (c_, e) in gfirst.items() if c_ == ci}
        call["last"] = {t: e for t, (c_, e) in glast.items() if c_ == ci}

    return dict(
        f=f, h=h, gsz=gsz, calls=calls, hot_rows=hot_rows, colds=colds,
        tokpad=tokpad, tokval=tokval, total_chunks=max(chunk_off, 1),
        total_entries=max(entry_off, 1),
    )


def _schedule(plan):
    """Greedy PE/Q7 co-sim; returns (sched, makespan_est)."""
    f, h = plan["f"], plan["h"]
    calls = plan["calls"]
    q7 = Q7_LIB_END_NS
    if h:
        q7 += 128 * h * Q7_NS_PER_IDX + Q7_CALL_FIXED_NS
    hot_land = q7 + DRAIN_NS
    call_end = []
    for call in calls:
        q7 += call["nch"] * P * Q7_NS_PER_IDX + Q7_CALL_FIXED_NS
        call_end.append(q7)
    land = [e + DRAIN_NS for e in call_end]

    pe = PE_START_NS
    sched = []
    fiat_left = list(range(NT)) if f else []
    fiat_done = set() if f else set(range(NT))
    consumed = []   # PE time when call i's matmuls done (gt slot free)
    ci = 0
    while ci < len(calls) or fiat_left:
        call_ready = ci < len(calls) and (not fiat_left or land[ci] <= pe)
        if call_ready or not fiat_left:
            call = calls[ci]
            for t in call["grp"]:
                if f and t not in fiat_done:
                    sched.append(("fiat", t))
                    fiat_done.add(t)
                    fiat_left.remove(t)
                    pe += f * PE_NS_PER_CHUNK + 100.0
            need_hot = len(call["first"]) * h
            start = max(pe, land[ci], hot_land if need_hot else 0.0)
            pe = start + (len(call["entries"]) + need_hot) * PE_NS_PER_CHUNK
            consumed.append(pe)
            sched.append(("call", ci))
            ci += 1
        else:
            t = fiat_left.pop(0)
            fiat_done.add(t)
            sched.append(("fiat", t))
            pe += f * PE_NS_PER_CHUNK + 100.0
    mk = max(pe + DVE_PASS_NS + 600.0,
             (call_end[-1] if call_end else q7) + 2 * PE_NS_PER_CHUNK
             + DVE_PASS_NS + 600.0) + TEARDOWN_NS
    return sched, mk


def _materialize(plan, uniqs, wmats):
    """Build idx/w/cfh numpy maps for each core."""
    f, h = plan["f"], plan["h"]
    FH = f + h
    calls = plan["calls"]
    hot_rows = plan["hot_rows"]
    total_chunks = plan["total_chunks"]
    total_entries = plan["total_entries"]
    wdt = ml_dtypes.bfloat16

    idx_all = np.zeros((NCORES, total_chunks * P), np.int64)
    w_all = np.zeros((NCORES, total_entries, P, P), np.float32)
    cfh_all = np.zeros((NCORES, NT, max(FH, 1), P, P), np.float32)
    for c in range(NCORES):
        if h > 0:
            idx_all[c, :hot_rows[c].size] = hot_rows[c]
        for t in range(NT):
            u = uniqs[c, t]
            wm = wmats[c, t]
            for j in range(f):
                lo, hi = j * P, (j + 1) * P
                sel = (u >= lo) & (u < hi)
                if sel.any():
                    cfh_all[c, t, j, u[sel] - lo] = wm[sel]
            hr = hot_rows[c]
            if h > 0:
                pos_in_u = np.minimum(np.searchsorted(u, hr), u.size - 1)
                ok = u[pos_in_u] == hr
                for j in range(h):
                    rows = np.arange(j * P, (j + 1) * P)
                    okj = ok[rows]
                    cfh_all[c, t, f + j, np.nonzero(okj)[0]] = (
                        wm[pos_in_u[rows[okj]]]
                    )
        for gidx, call in enumerate(calls):
            toks_l = plan["tokpad"][c, gidx]
            valid = plan["tokval"][c, gidx]
            b0 = call["idx_base"]
            idx_all[c, b0 * P:b0 * P + toks_l.size] = toks_l
            in_t = {
                t: valid & np.isin(toks_l, plan["colds"][c, t])
                for t in call["grp"]
            }
            for e, (j, tt) in enumerate(call["entries"]):
                lo, hi = j * P, (j + 1) * P
                seg = toks_l[lo:hi]
                side = in_t[tt][lo:hi]
                if not side.any():
                    continue
                uu = uniqs[c, tt]
                rows = np.searchsorted(uu, seg[side])
                w_all[c, call["entry_base"] + e, np.nonzero(side)[0]] = (
                    wmats[c, tt][rows]
                )

    idx_maps, w_maps, cff_maps, cfh_maps = [], [], [], []
    for c in range(NCORES):
        idxw = np.tile(idx_all[c].reshape(-1, 16).T, (8, 1)).astype(np.int16)
        idx_maps.append(np.ascontiguousarray(idxw))
        wf = w_all[c].transpose(1, 0, 2).reshape(P, -1).astype(wdt)
        w_maps.append(np.ascontiguousarray(wf))
        cfp = cfh_all[c][:, :max(f, 1)] if f else cfh_all[c][:, :1]
        chp = cfh_all[c][:, f:f + h] if h else cfh_all[c][:, :1]
        cff_maps.append(np.ascontiguousarray(
            cfp.reshape(NT * max(f, 1), P, P).transpose(1, 0, 2)
            .reshape(P, -1).astype(wdt)))
        cfh_maps.append(np.ascontiguousarray(
            chp.reshape(NT * max(h, 1), P, P).transpose(1, 0, 2)
            .reshape(P, -1).astype(wdt)))
    return idx_maps, w_maps, cff_maps, cfh_maps


def _prepare(phon_tokens, group_len_raw):
    uniqs, wmats, kvs = _base_stats(phon_tokens, group_len_raw)
    fe = os.environ.get("F")
    he = os.environ.get("H")
    ge = os.environ.get("GSZ")
    if fe is not None and he is not None:
        grid = [(int(fe), int(he), int(ge or 2))]
    else:
        # (3, 3, pairs) measured fastest on HW; the rest are fallbacks in
        # case a different input distribution breaks its chunk-count asserts
        grid = [(3, 3, 2)] + [
            (ff, hh, gg)
            for ff in (1, 2, 3)
            for hh in (2, 3, 4, 5)
            for gg in (2, 4)
        ]
    best = None
    for (ff, hh, gg) in grid:
        try:
            plan = _plan(uniqs, kvs, ff, hh, gg)
        except AssertionError:
            continue
        sched, mk = _schedule(plan)
        if best is None or mk < best[0]:
            best = (mk, plan, sched)
        if (ff, hh, gg) == (3, 3, 2):
            break  # preferred config planned successfully
    mk, plan, sched = best
    if os.environ.get("VERBOSE"):
        nidx = 128 * plan["h"] + sum(
            c["nch"] * P for c in plan["calls"])
        nent = sum(len(c["entries"]) for c in plan["calls"])
        print(f"[plan] f={plan['f']} h={plan['h']} gsz={plan['gsz']} "
              f"makespan={mk/1000:.1f}us idx={nidx} entries={nent}")
    idx_maps, w_maps, cff_maps, cfh_maps = _materialize(plan, uniqs, wmats)
    meta = dict(plan=plan, sched=sched)
    return meta, idx_maps, w_maps, cff_maps, cfh_maps


def _build_nc(meta):
    mdt = mybir.dt.bfloat16
    f32 = mybir.dt.float32
    plan = meta["plan"]
    sched = meta["sched"]
    f, h = plan["f"], plan["h"]
    FH = f + h
    calls = plan["calls"]
    total_chunks = plan["total_chunks"]
    total_entries = plan["total_entries"]
    max_entries = max((len(c["entries"]) for c in calls), default=1)
    max_nch = max((c["nch"] for c in calls), default=1)

    nc = bacc.Bacc("TRN2", target_bir_lowering=False, debug=False)

    table_d = nc.dram_tensor("table", [VOCAB, D], mdt, kind="ExternalInput")
    tablek_d = nc.dram_tensor("tablek", [P, KT * D], mdt, kind="ExternalInput")
    pos_d = nc.dram_tensor("pos", [P, D], f32, kind="ExternalInput")
    idx_d = nc.dram_tensor("idxs", [P, total_chunks * 8], mybir.dt.int16,
                           kind="ExternalInput")
    w_d = nc.dram_tensor("wmat", [P, total_entries * P], mdt,
                         kind="ExternalInput")
    cff_d = nc.dram_tensor("cff", [P, NT * max(f, 1) * P], mdt,
                           kind="ExternalInput")
    cfh_d = nc.dram_tensor("cfh", [P, NT * max(h, 1) * P], mdt,
                           kind="ExternalInput")
    out_d = nc.dram_tensor("out", [R, D], mdt, kind="ExternalOutput")

    with tile.TileContext(nc) as tc:
        with (
            tc.tile_pool(name="const", bufs=1) as cpool,
            tc.tile_pool(name="gather", bufs=GT_BUFS) as gpool,
            tc.tile_pool(name="wpool", bufs=3) as wpool,
            tc.tile_pool(name="osb", bufs=4) as opool,
            tc.tile_pool(name="psum", bufs=8, space=bass.MemorySpace.PSUM) as ppool,
        ):
            idx_sb = cpool.tile([P, total_chunks * 8], mybir.dt.int16)
            # hot gather first-emitted: its (auto-inserted) library load
            # starts the moment the gpsimd preamble ends.
            nregs = {}

            def _nreg(n):
                if n not in nregs:
                    nregs[n] = nc.gpsimd.to_reg(n)
                return nregs[n]

            nc.scalar.dma_start(idx_sb[:], idx_d[:])
            hot_sb = None
            if h:
                hot_sb = cpool.tile([P, h, D], mdt)
                nc.gpsimd.dma_gather(
                    hot_sb[:, :, :], table_d[:], idx_sb[:, :h * 8],
                    num_idxs=h * P, num_idxs_reg=_nreg(h * P), elem_size=D,
                )
            # load order tuned for the DMA-bandwidth-starved early window
            # (the Q7 library load shares HBM bandwidth until ~21.8us): the
            # first fiat burst needs only cff tile 0 + fiat chunk 0.
            # spread upfront loads across engine DMA queues: sync carries
            # only the first-matmul critical path; idle scalar/vector queues
            # issue the rest concurrently.
            cff_sb = None
            fiat_sb = None
            if f:
                cff_sb = cpool.tile([P, NT, f * P], mdt)
                nc.sync.dma_start(cff_sb[:, 0:1, :], cff_d[:, :f * P])
                fiat_sb = cpool.tile([P, f, D], mdt)
                # halves: the very first N=512 matmul needs only 128KB
                nc.sync.dma_start(fiat_sb[:, 0, :512], tablek_d[:, :512])
                nc.sync.dma_start(fiat_sb[:, 0, 512:], tablek_d[:, 512:D])
                nc.scalar.dma_start(cff_sb[:, 1:2, :],
                                    cff_d[:, f * P:2 * f * P])
                if f > 1:
                    # needed by the first burst's j>=1 matmuls: before pos
                    nc.sync.dma_start(
                        fiat_sb[:, 1:, :], tablek_d[:, D:f * D]
                    )
            pos_sb = cpool.tile([P, D], f32)
            nc.sync.dma_start(pos_sb[:], pos_d[:])
            if f:
                nc.scalar.dma_start(cff_sb[:, 2:4, :],
                                    cff_d[:, 2 * f * P:4 * f * P])
                for lo, hi in ((4, 8), (8, 12), (12, 16)):
                    nc.scalar.dma_start(
                        cff_sb[:, lo:hi, :],
                        cff_d[:, lo * f * P:hi * f * P],
                    )
            cfh_sb = None
            if h:
                cfh_sb = cpool.tile([P, NT, h * P], mdt)
                for lo, hi in ((0, 4), (4, 8), (8, 16)):
                    nc.scalar.dma_start(
                        cfh_sb[:, lo:hi, :],
                        cfh_d[:, lo * h * P:hi * h * P],
                    )

            out_sb = cpool.tile([P, NT, D], mdt)
            psums = {}

            for kind, item in sched:
                if kind == "fiat":
                    t = item
                    ps2 = (ppool.tile([P, 512], f32, tag="ps", name="psa"),
                           ppool.tile([P, 512], f32, tag="ps", name="psb"))
                    for j in range(f):
                        for hh in range(0, D, 512):
                            nc.tensor.matmul(
                                ps2[hh // 512][:, :],
                                lhsT=cff_sb[:, t, j * P:(j + 1) * P],
                                rhs=fiat_sb[:, j, hh:hh + 512],
                                start=(j == 0), stop=(j == f - 1),
                            )
                    for hh in range(0, D, 512):
                        nc.vector.tensor_tensor(
                            out_sb[:, t, hh:hh + 512], ps2[hh // 512][:, :],
                            pos_sb[:, hh:hh + 512],
                            op=mybir.AluOpType.add,
                        )
                else:
                    call = calls[item]
                    nch = call["nch"]
                    b0 = call["idx_base"]
                    n_idx = nch * P
                    gt = gpool.tile([P, max_nch, D], mdt, tag="gt")
                    nc.gpsimd.dma_gather(
                        gt[:, :nch, :], table_d[:],
                        idx_sb[:, b0 * 8:(b0 + nch) * 8],
                        num_idxs=n_idx, num_idxs_reg=_nreg(n_idx),
                        elem_size=D,
                    )
                    ne = len(call["entries"])
                    wt = wpool.tile([P, max_entries * P], mdt, tag="wt")
                    eb = call["entry_base"]
                    if ne:
                        nc.scalar.dma_start(
                            wt[:, :ne * P], w_d[:, eb * P:(eb + ne) * P]
                        )
                    for e, (j, t) in enumerate(call["entries"]):
                        if call["first"].get(t, -1) == e:
                            ps2 = (ppool.tile([P, 512], f32, tag="ps", name="psa"),
                                   ppool.tile([P, 512], f32, tag="ps", name="psb"))
                            psums[t] = ps2
                            for jj in range(h):
                                for hh in range(0, D, 512):
                                    nc.tensor.matmul(
                                        ps2[hh // 512][:, :],
                                        lhsT=cfh_sb[
                                            :, t, jj * P:(jj + 1) * P,
                                        ],
                                        rhs=hot_sb[:, jj, hh:hh + 512],
                                        start=(jj == 0), stop=False,
                                    )
                        first_mm = (h == 0 and call["first"].get(t, -1) == e)
                        for hh in range(0, D, 512):
                            nc.tensor.matmul(
                                psums[t][hh // 512][:, :],
                                lhsT=wt[:, e * P:(e + 1) * P],
                                rhs=gt[:, j, hh:hh + 512],
                                start=first_mm,
                                stop=(call["last"].get(t, -1) == e),
                            )
                        if call["last"].get(t, -1) == e:
                            ot = opool.tile([P, D], mdt, tag="ot")
                            for hh in range(0, D, 512):
                                nc.vector.tensor_tensor(
                                    ot[:, hh:hh + 512],
                                    psums[t][hh // 512][:, :],
                                    out_sb[:, t, hh:hh + 512] if f
                                    else pos_sb[:, hh:hh + 512],
                                    op=mybir.AluOpType.add,
                                )
                            nc.sync.dma_start(
                                out_d[t * P:(t + 1) * P, :], ot[:]
                            )
    nc.compile()
    return nc


def run(inputs, trace=False, tmpdir=None):
    meta, idx_maps, w_maps, cff_maps, cfh_maps = _prepare(
        inputs["phon_tokens"], inputs["group_len_raw"]
    )
    wdt = ml_dtypes.bfloat16
    table_np = np.ascontiguousarray(
        np.asarray(inputs["phon_emb_table"]).astype(wdt)
    )
    tablek_np = np.ascontiguousarray(
        table_np.reshape(KT, P, D).transpose(1, 0, 2).reshape(P, KT * D)
    )
    pos_np = np.ascontiguousarray(
        np.asarray(inputs["pos_emb_table"]).astype(np.float32)
    )

    nc = _build_nc(meta)
    in_maps = [
        {
            "table": table_np, "tablek": tablek_np, "pos": pos_np,
            "idxs": idx_maps[c], "wmat": w_maps[c],
            "cff": cff_maps[c], "cfh": cfh_maps[c],
        }
        for c in range(NCORES)
    ]
    res = run_bass_kernel_spmd(
        nc, in_maps, core_ids=list(range(NCORES)), trace=trace, tmpdir=tmpdir
    )
    out = np.empty((B, S, D), np.float32)
    for c in range(NCORES):
        out[c * BPC:(c + 1) * BPC] = (
            res.results[c]["out"].astype(np.float32).reshape(BPC, S, D)
        )
    return out, res


def kernel(**inputs) -> np.ndarray:
    out, _ = run(inputs, trace=False)
    return out

